# revision 10
# baseline (speedup 1.0000x reference)
"""Trainium2 Bass kernel for nn_ADIAModel (dense_transformer).

Data-parallel over batch B=16 across 8 NeuronCores (2 batches/core).
Per core: conv stack (stem + 5 residual ConvBlocks, bf16 matmuls, f32 PSUM,
fused groupnorm via bn_stats + PE group-reduce), then 2 transformer layers
over 132 edge tokens in transposed [feature, token] layout with matmul-trick
LayerNorms, polynomial struct-bias gather, and no-max softmax (scores are
bounded, verified |s| < 3).

Self-contained: hardcodes all shapes; builds + compiles + runs on devices
0..7 via run_bass_kernel_spmd; returns full (edge_logits, node_logits).
"""
import os
import numpy as np
import ml_dtypes

import concourse.bass as bass
import concourse.mybir as mybir
import concourse.tile as tile
from concourse import bacc
from concourse.bass_utils import run_bass_kernel_spmd

F32 = mybir.dt.float32
BF16 = mybir.dt.bfloat16
BF = ml_dtypes.bfloat16
AF = mybir.ActivationFunctionType
OP = mybir.AluOpType

# model dims
P_COLS = 12
E = 132            # edges/tokens per batch
B, C_IN, N = 16, 8, 512
D, H, HD, NG = 64, 4, 16, 8
SCALE = HD ** -0.5
NCORES = 8
BPC = B // NCORES   # 2 batches per core
M = BPC * E         # 264 samples per core
NPAIR = M // 2      # 132
GROUP = 5           # pairs per stats group (5 psum y-banks + 2 stat banks)
NL = 5              # conv layers
FF = 4 * D

_uid = [0]


def _nm(p):
    _uid[0] += 1
    return f"{p}{_uid[0]}"


def _lagrange6(vals):
    V = np.vander(np.arange(6.0), 6, increasing=True)
    return np.linalg.solve(V, np.asarray(vals, np.float64))


def _emit_ln(nc, wk, pp, x_sb, w, gamma_ap, beta_ap, gelu, ones64, ones164, eps1):
    """LayerNorm over the 64 partitions of x_sb [64, w] -> new [64, w] tile.
    Mean/var via PE ones-matmuls; apply via DVE; affine+act fused on ACT."""
    sq = wk.tile([64, w], F32, tag=f"ln_sq{w}", name=_nm("lnsq"), bufs=2)
    nc.scalar.activation(out=sq, in_=x_sb, func=AF.Square)
    ps_s = pp.tile([1, w], F32, tag="big", name=_nm("lns"), bufs=2)
    ps_q = pp.tile([1, w], F32, tag="big", name=_nm("lnq"), bufs=2)
    nc.tensor.matmul(ps_s, ones64, x_sb, start=True, stop=True)
    nc.tensor.matmul(ps_q, ones64, sq, start=True, stop=True)
    mu = wk.tile([1, w], F32, tag=f"ln_mu{w}", name=_nm("lnmu"), bufs=2)
    nc.vector.tensor_copy(out=mu, in_=ps_s)
    var = wk.tile([1, w], F32, tag=f"ln_var{w}", name=_nm("lnvar"), bufs=2)
    nc.vector.tensor_mul(out=var, in0=mu, in1=mu)
    nc.vector.tensor_sub(out=var, in0=ps_q, in1=var)
    sd = wk.tile([1, w], F32, tag=f"ln_sd{w}", name=_nm("lnsd"), bufs=2)
    nc.scalar.activation(out=sd, in_=var, func=AF.Sqrt, bias=eps1[0:1, :])
    rs = wk.tile([1, w], F32, tag=f"ln_rs{w}", name=_nm("lnrs"), bufs=2)
    nc.vector.reciprocal(out=rs, in_=sd)
    ps_b1 = pp.tile([64, w], F32, tag="big", name=_nm("lnb1"), bufs=2)
    ps_b2 = pp.tile([64, w], F32, tag="big", name=_nm("lnb2"), bufs=2)
    nc.tensor.matmul(ps_b1, ones164, mu, start=True, stop=True)
    nc.tensor.matmul(ps_b2, ones164, rs, start=True, stop=True)
    t1 = wk.tile([64, w], F32, tag=f"ln_t{w}", name=_nm("lnt"), bufs=2)
    nc.vector.tensor_sub(out=t1, in0=x_sb, in1=ps_b1)
    nc.vector.tensor_mul(out=t1, in0=t1, in1=ps_b2)
    out = wk.tile([64, w], F32, tag=f"ln_o{w}", name=_nm("lno"), bufs=2)
    nc.scalar.activation(out=out, in_=t1, func=AF.Gelu if gelu else AF.Identity,
                         scale=gamma_ap, bias=beta_ap)
    return out


def build_nc(pp_np):
    """pp_np: host-prepped replicated parameter arrays (dict name->np array)
    plus 'polyco' [2,4,6] float64 struct-bias polynomial coefficients."""
    nc = bacc.Bacc("TRN2", target_bir_lowering=False)

    dp = {}
    for name, arr in pp_np.items():
        if name == "polyco":
            continue
        dt = BF16 if arr.dtype == BF else F32
        dp[name] = nc.declare_dram_parameter(name, list(arr.shape), dt, isOutput=False)
    edb = nc.declare_dram_parameter("edb", [M, C_IN, N], BF16, isOutput=False)
    typesf = nc.declare_dram_parameter("typesf", [1, M], F32, isOutput=False)
    maskk = nc.declare_dram_parameter("maskk", [BPC, E, 1], F32, isOutput=False)
    relt = nc.declare_dram_parameter("relt", [BPC, E, E], F32, isOutput=False)
    edge_out = nc.declare_dram_parameter("edge_out", [2, M], F32, isOutput=True)
    node_out = nc.declare_dram_parameter("node_out", [8, 2 * (P_COLS - 2)], F32, isOutput=True)
    polyco = pp_np["polyco"]

    dbg_names = [s for s in os.environ.get("BASSK_DEBUG", "").split(",") if s]
    dbg_outs = {}

    def dbg(name, ap, shape):
        if name not in dbg_names:
            return
        d = nc.declare_dram_parameter("dbg_" + name, list(shape), F32, isOutput=True)
        dbg_outs[name] = d
        nc.sync.dma_start(out=d[:], in_=ap)

    npair = int(os.environ.get("BASSK_PAIRS", NPAIR))

    with tile.TileContext(nc) as tc:
        with tc.tile_pool(name="cst", bufs=1) as cst, \
             tc.tile_pool(name="wk", bufs=2) as wk:
            # ---- load constants/weights into SBUF
            c = {}
            for name, arr in pp_np.items():
                if name == "polyco":
                    continue
                dt = BF16 if arr.dtype == BF else F32
                c[name] = cst.tile(list(arr.shape), dt, tag=name, name=_nm(name))
                nc.sync.dma_start(out=c[name], in_=dp[name][:])
            eps1 = cst.tile([16, 1], F32, tag="eps1", name="eps1")
            nc.vector.memset(eps1, 1e-5)

            sh64 = c["sheet64"]

            # ---- struct-bias tiles via degree-5 polynomial of rel (per layer/batch/chunk/head)
            # chunks over k: [0:128] and [128:132]
            CH = [(0, 128), (128, 4)]
            rel_sb = {}
            bias_sb = {}
            for b in range(BPC):
                for ci, (off, kc) in enumerate(CH):
                    r = wk.tile([kc, E], F32, tag=f"rel{ci}", name=_nm("rel"), bufs=2)
                    nc.sync.dma_start(out=r, in_=relt[b, off:off + kc, :])
                    rel_sb[(b, ci)] = r
                    pw = [r]
                    for k, (i0, i1) in enumerate([(0, 0), (1, 0), (1, 1), (2, 1)]):
                        rk = wk.tile([kc, E], F32, tag=f"rpw{ci}_{k}", name=_nm("rpw"), bufs=2)
                        nc.vector.tensor_mul(out=rk, in0=pw[i0], in1=pw[i1])
                        pw.append(rk)
                    # pw = [r, r2, r3, r4, r5]: r2=r*r, r3=r2*r, r4=r2*r2, r5=r3*r2
                    for l in range(2):
                        for h in range(H):
                            a = polyco[l][h]
                            bt = wk.tile([kc, E], F32, tag=f"bias{b}_{ci}_{l}_{h}",
                                         name=_nm("bias"), bufs=1)
                            nc.vector.tensor_scalar(out=bt, in0=r, scalar1=float(a[1]),
                                                    scalar2=float(a[0]), op0=OP.mult, op1=OP.add)
                            for k in range(2, 6):
                                nc.vector.scalar_tensor_tensor(
                                    out=bt, in0=pw[k - 1], scalar=float(a[k]), in1=bt,
                                    op0=OP.mult, op1=OP.add)
                            bias_sb[(l, b, ci, h)] = bt

            mk_sb = {}
            for b in range(BPC):
                for ci, (off, kc) in enumerate(CH):
                    mk = cst.tile([kc, 1], F32, tag=f"mk{b}_{ci}", name=_nm("mk"))
                    nc.sync.dma_start(out=mk, in_=maskk[b, off:off + kc, :])
                    mk_sb[(b, ci)] = mk

            # ---- conv stack ----
            stage = cst.tile([128, NPAIR], F32, tag="stage", name="stage")
            with tc.tile_pool(name="cvp", bufs=1, space="PSUM") as cvp, \
                 tc.tile_pool(name="cvs", bufs=2) as cvs:
                ngroups = (npair + GROUP - 1) // GROUP
                for gi in range(ngroups):
                    prs = list(range(gi * GROUP, min((gi + 1) * GROUP, npair)))
                    G = len(prs)
                    xt = {}
                    yp = {}
                    act = {}
                    edg = cvs.tile([128, GROUP, N], BF16, tag="edg", name=_nm("edg"), bufs=2)
                    p0 = prs[0]
                    nc.sync.dma_start(
                        out=edg[0:C_IN, 0:G, :],
                        in_=edb[2 * p0:2 * p0 + 2 * G:2].rearrange("j c n -> c j n"))
                    nc.sync.dma_start(
                        out=edg[64:64 + C_IN, 0:G, :],
                        in_=edb[2 * p0 + 1:2 * p0 + 2 * G:2].rearrange("j c n -> c j n"))
                    for j, p in enumerate(prs):
                        x = cvs.tile([128, N + 4], BF16, tag=f"x{j}", name=_nm("x"), bufs=2)
                        if gi < 2:
                            nc.vector.memset(x[:, 0:2], 0.0)
                            nc.vector.memset(x[:, N + 2:N + 4], 0.0)
                        ac = cvs.tile([128, 8], F32, tag=f"acc{j}", name=_nm("acc"), bufs=2)
                        ps = cvp.tile([128, N], F32, tag=f"y{j}", name=_nm("yst"), bufs=1)
                        nc.tensor.matmul(ps[0:64, :], c["wstem"][0:8, :], edg[0:8, j, :],
                                         start=True, stop=True, tile_position=(0, 0))
                        nc.tensor.matmul(ps[64:128, :], c["wstem"][64:72, :], edg[64:72, j, :],
                                         start=True, stop=True, tile_position=(64, 64))
                        nc.scalar.activation(out=x[:, 2:N + 2], in_=ps, func=AF.Identity,
                                             bias=c["bstem"][:, 0:1], accum_out=ac[:, 0:1])
                        act[j] = ac
                        if gi == 0 and j == 0 and dbg_names:
                            xf = wk.tile([128, N], F32, tag="dbgxf", name=_nm("dbgxf"))
                            nc.vector.tensor_copy(out=xf, in_=x[:, 2:N + 2])
                            dbg("x_stem", xf[:], [128, N])
                        xt[j] = x
                        yp[j] = ps
                    for l in range(NL):
                        st6 = cvs.tile([128, G, 6], F32, tag="st6", name=_nm("st6"), bufs=2)
                        mv = cvs.tile([128, G, 2], F32, tag="mv", name=_nm("mv"), bufs=2)
                        for j, p in enumerate(prs):
                            ps = yp[j]
                            x = xt[j]
                            for t in range(3):
                                nc.tensor.matmul(
                                    ps[0:64, :], c["wconv"][0:64, l, t, :],
                                    xt[j][0:64, 1 + t:1 + t + N],
                                    start=(t == 0), stop=(t == 2), tile_position=(0, 0))
                            for t in range(3):
                                nc.tensor.matmul(
                                    ps[64:128, :], c["wconv"][64:128, l, t, :],
                                    xt[j][64:128, 1 + t:1 + t + N],
                                    start=(t == 0), stop=(t == 2), tile_position=(64, 64))
                            nc.vector.bn_stats(out=st6[:, j, :], in_=ps)
                            nc.vector.bn_aggr(out=mv[:, j, :], in_=st6[:, j, :])
                            if gi == 0 and l == 0 and j == 0 and dbg_names:
                                yf = wk.tile([128, N], F32, tag="dbgyf", name=_nm("dbgyf"))
                                nc.vector.tensor_copy(out=yf, in_=ps)
                                dbg("y00", yf[:], [128, N])
                        # group stats chain
                        se = cvs.tile([128, G, 2], F32, tag="se", name=_nm("se"), bufs=2)
                        nc.vector.tensor_copy(out=se[:, :, 0], in_=mv[:, :, 0])
                        nc.vector.tensor_mul(out=se[:, :, 1], in0=mv[:, :, 0], in1=mv[:, :, 0])
                        nc.vector.tensor_add(out=se[:, :, 1], in0=se[:, :, 1], in1=mv[:, :, 1])
                        gs = cvp.tile([16, 2 * G], F32, tag="sp0", name=_nm("gs"), bufs=1)
                        nc.tensor.matmul(gs, c["g8"], se[:].rearrange("p a b -> p (a b)"),
                                         start=True, stop=True)
                        gsb = cvs.tile([16, G, 2], F32, tag="gsb", name=_nm("gsb"), bufs=2)
                        nc.vector.tensor_copy(out=gsb[:].rearrange("p a b -> p (a b)"), in_=gs)
                        m2g = cvs.tile([16, G], F32, tag="m2g", name=_nm("m2g"), bufs=2)
                        nc.vector.tensor_mul(out=m2g, in0=gsb[:, :, 0], in1=gsb[:, :, 0])
                        gv = cvs.tile([16, G], F32, tag="gv", name=_nm("gv"), bufs=2)
                        nc.vector.tensor_sub(out=gv, in0=gsb[:, :, 1], in1=m2g)
                        # rstd = rsqrt(gv + eps) via Quake seed + 2 Newton iters (DVE only,
                        # avoids ACT table switch between Sqrt and Gelu sets)
                        nc.vector.tensor_scalar(out=gv, in0=gv, scalar1=1e-5, scalar2=None,
                                                op0=OP.add)
                        zi = cvs.tile([16, G], mybir.dt.int32, tag="zi", name=_nm("zi"), bufs=2)
                        nc.vector.tensor_scalar(out=zi, in0=gv[:].bitcast(mybir.dt.int32),
                                                scalar1=1, scalar2=None,
                                                op0=OP.arith_shift_right)
                        nc.vector.tensor_scalar(out=zi, in0=zi, scalar1=-1,
                                                scalar2=0x5f3759df, op0=OP.mult, op1=OP.add)
                        z = zi[:].bitcast(F32)
                        w_ = cvs.tile([16, G], F32, tag="w_", name=_nm("w_"), bufs=2)
                        for _ in range(2):
                            nc.vector.tensor_mul(out=w_, in0=z, in1=z)
                            nc.vector.tensor_mul(out=w_, in0=w_, in1=gv)
                            nc.vector.tensor_scalar(out=w_, in0=w_, scalar1=-0.5,
                                                    scalar2=1.5, op0=OP.mult, op1=OP.add)
                            nc.vector.tensor_mul(out=zi[:].bitcast(F32), in0=z, in1=w_)
                        gb = cvs.tile([16, G, 2], F32, tag="gb", name=_nm("gb"), bufs=2)
                        nc.vector.tensor_copy(out=gb[:, :, 0], in_=gsb[:, :, 0])
                        nc.vector.tensor_copy(out=gb[:, :, 1], in_=z)
                        bc = cvp.tile([128, 2 * G], F32, tag="sp1", name=_nm("bc"), bufs=1)
                        nc.tensor.matmul(bc, c["g8t"], gb[:].rearrange("p a b -> p (a b)"),
                                         start=True, stop=True)
                        bcv = bc[:].rearrange("p (a b) -> p a b", b=2)
                        stg = cvs.tile([128, G, 2], F32, tag="stg", name=_nm("stg"), bufs=2)
                        nc.vector.tensor_scalar(out=stg[:, :, 0], in0=bcv[:, :, 1],
                                                scalar1=c["gnw"][:, l, 0:1], scalar2=None,
                                                op0=OP.mult)
                        tmpg = cvs.tile([128, G], F32, tag="tmpg", name=_nm("tmpg"), bufs=2)
                        nc.vector.tensor_mul(out=tmpg, in0=bcv[:, :, 0], in1=stg[:, :, 0])
                        nc.vector.tensor_scalar(out=stg[:, :, 1], in0=tmpg,
                                                scalar1=-1.0, scalar2=c["gnw"][:, l, 1:2],
                                                op0=OP.mult, op1=OP.add)
                        if gi == 0 and l == 0 and dbg_names:
                            dbg("mv0", mv[:].rearrange("p a b -> p (a b)"), [128, 2 * G])
                            dbg("gb0", gb[:].rearrange("p a b -> p (a b)"), [16, 2 * G])
                            dbg("stg0", stg[:].rearrange("p a b -> p (a b)"), [128, 2 * G])
                        for j, p in enumerate(prs):
                            g = cvs.tile([128, N], BF16, tag="g", name=_nm("g"), bufs=3)
                            nc.scalar.activation(out=g, in_=yp[j], func=AF.Gelu,
                                                 scale=stg[:, j, 0:1], bias=stg[:, j, 1:2],
                                                 accum_out=act[j][:, l + 1:l + 2])
                            eng = nc.vector if (p % 2 == 0) else nc.gpsimd
                            eng.tensor_add(out=xt[j][:, 2:N + 2], in0=xt[j][:, 2:N + 2], in1=g)
                    for j, p in enumerate(prs):
                        nc.vector.reduce_sum(out=stage[:, p:p + 1], in_=act[j][:, 0:NL + 1],
                                             axis=mybir.AxisListType.X)

            dbg("stage", stage[:], [128, NPAIR])
            dbg("bias000", bias_sb[(0, 0, 0, 0)][:], [128, E])
            # ---- merge + transformer (separate PSUM pool) ----
            with tc.tile_pool(name="apq", bufs=2, space="PSUM") as pp:
                convT = cst.tile([64, M], F32, tag="convT", name="convT")
                sv = stage[:].rearrange("(s c) p -> s c p", s=2)
                cv = convT[:].rearrange("c (p s) -> c s p", s=2)
                nc.sync.dma_start(out=cv[:, 0, :], in_=sv[0])
                nc.sync.dma_start(out=cv[:, 1, :], in_=sv[1])

                # type embedding via one-hot matmul
                tb = wk.tile([7, M], F32, tag="tb", name="tb")
                nc.sync.dma_start(out=tb, in_=bass.AP(tensor=typesf[:].tensor, offset=0,
                                                      ap=[[0, 7], [1, M]]))
                oh = wk.tile([7, M], F32, tag="oh", name="oh")
                nc.vector.tensor_scalar(out=oh, in0=tb, scalar1=c["iota7"][:, 0:1],
                                        scalar2=None, op0=OP.is_equal)
                ps_t = pp.tile([64, M], F32, tag="big", name=_nm("pst"), bufs=2)
                nc.tensor.matmul(ps_t, c["te"], oh, start=True, stop=True)
                typT = wk.tile([64, M], F32, tag="typT", name="typT")
                nc.vector.tensor_copy(out=typT, in_=ps_t)
                dbg("convT", convT[:], [64, M])
                dbg("typT", typT[:], [64, M])

                ps_e = pp.tile([64, M], F32, tag="big", name=_nm("pse"), bufs=2)
                nc.tensor.matmul(ps_e, c["wm"][:, 0, :], convT, start=True, stop=False)
                nc.tensor.matmul(ps_e, c["wm"][:, 1, :], typT, start=False, stop=True)
                el = wk.tile([64, M], F32, tag="el", name="el")
                nc.scalar.activation(out=el, in_=ps_e, func=AF.Identity, bias=sh64[:, 0:1])
                dbg("el", el[:], [64, M])
                eT = _emit_ln(nc, wk, pp, el, M, sh64[:, 1:2], sh64[:, 2:3], True,
                              c["ones64"], c["ones164"], eps1)
                dbg("eT0", eT[:], [64, M])

                # ---- 2 attention layers ----
                for l in range(2):
                    base = 3 + l * 9
                    prj = {}
                    for j, pname in enumerate(["q", "k", "v"]):
                        psp = pp.tile([128, M], F32, tag="big", name=_nm("psp"), bufs=2)
                        nc.tensor.matmul(psp, c["wqkv"][:, l * 3 + j, :], eT,
                                         start=True, stop=True)
                        pj = wk.tile([128, M], F32, tag=f"prj{pname}", name=_nm("prj"))
                        nc.vector.tensor_scalar(out=pj, in0=psp,
                                                scalar1=c["bqkv"][:, l * 3 + j, :],
                                                scalar2=None, op0=OP.add)
                        prj[pname] = pj
                    # token-major V per (batch, chunk) via PE transpose
                    vtm = {}
                    for b in range(BPC):
                        for ci, (off, kc) in enumerate(CH):
                            pv = pp.tile([kc, 128], F32, tag="sc", name=_nm("pv"), bufs=2)
                            nc.tensor.transpose(pv, prj["v"][:, b * E + off:b * E + off + kc],
                                                c["id128"])
                            vt = wk.tile([kc, 128], F32, tag=f"vtm{ci}", name=_nm("vtm"), bufs=2)
                            nc.vector.tensor_copy(out=vt, in_=pv)
                            vtm[(b, ci)] = vt
                    ocat = {}
                    for b in range(BPC):
                        ps_dn = pp.tile([4, E], F32, tag="dn", name=_nm("dn"), bufs=2)
                        ps_oT = pp.tile([128, E], F32, tag="oT", name=_nm("oT"), bufs=2)
                        first_dn = True
                        for ci, (off, kc) in enumerate(CH):
                            for h in range(H):
                                ps_sc = pp.tile([kc, E], F32, tag="sc", name=_nm("sc"), bufs=2)
                                nc.tensor.matmul(
                                    ps_sc,
                                    prj["k"][32 * h:32 * h + 16, b * E + off:b * E + off + kc],
                                    prj["q"][32 * h:32 * h + 16, b * E:(b + 1) * E],
                                    start=True, stop=True, tile_position=(32 * h, 0))
                                ex = wk.tile([kc, E], F32, tag=f"ex{ci}", name=_nm("ex"), bufs=3)
                                nc.vector.tensor_add(out=ex, in0=ps_sc,
                                                     in1=bias_sb[(l, b, ci, h)])
                                nc.scalar.activation(out=ex, in_=ex, func=AF.Exp)
                                nc.vector.tensor_scalar(out=ex, in0=ex,
                                                        scalar1=mk_sb[(b, ci)],
                                                        scalar2=None, op0=OP.mult)
                                nc.tensor.matmul(ps_dn, c["sel4"][0:kc, 4 * h:4 * h + 4], ex,
                                                 start=first_dn, stop=(ci == 1 and h == H - 1))
                                first_dn = False
                                nc.tensor.matmul(ps_oT[32 * h:32 * h + 16, :],
                                                 vtm[(b, ci)][:, 32 * h:32 * h + 16], ex,
                                                 start=(ci == 0), stop=(ci == 1),
                                                 tile_position=(0, 32 * h))
                        rd = wk.tile([4, E], F32, tag="rd", name=_nm("rd"), bufs=2)
                        nc.vector.reciprocal(out=rd, in_=ps_dn)
                        ps_rn = pp.tile([128, E], F32, tag="sc", name=_nm("rn"), bufs=2)
                        nc.tensor.matmul(ps_rn, c["ind4"], rd, start=True, stop=True)
                        rn = wk.tile([128, E], F32, tag="rn", name=_nm("rnsb"), bufs=2)
                        nc.vector.tensor_copy(out=rn, in_=ps_rn)
                        oc = wk.tile([128, E], F32, tag=f"oc{b}", name=_nm("oc"), bufs=2)
                        nc.vector.tensor_mul(out=oc, in0=ps_oT, in1=rn)
                        ocat[b] = oc
                    ps_o = pp.tile([64, M], F32, tag="big", name=_nm("pso"), bufs=2)
                    for b in range(BPC):
                        nc.tensor.matmul(ps_o[:, b * E:(b + 1) * E], c["wo"][:, l, :],
                                         ocat[b], start=True, stop=True)
                    o_sb = wk.tile([64, M], F32, tag="osb", name=_nm("osb"))
                    nc.vector.tensor_scalar(out=o_sb, in0=ps_o, scalar1=sh64[:, base:base + 1],
                                            scalar2=None, op0=OP.add)
                    e1 = wk.tile([64, M], F32, tag="e1", name=_nm("e1"))
                    nc.vector.tensor_add(out=e1, in0=eT, in1=o_sb)
                    eL = _emit_ln(nc, wk, pp, e1, M, sh64[:, base + 1:base + 2],
                                  sh64[:, base + 2:base + 3], False,
                                  c["ones64"], c["ones164"], eps1)
                    # FF
                    f1 = {}
                    for half in range(2):
                        ps_f = pp.tile([128, M], F32, tag="big", name=_nm("psf"), bufs=2)
                        nc.tensor.matmul(ps_f, c["w1"][:, l, 128 * half:128 * (half + 1)],
                                         eL, start=True, stop=True)
                        fh = wk.tile([128, M], F32, tag=f"f1{half}", name=_nm("f1"))
                        nc.scalar.activation(out=fh, in_=ps_f, func=AF.Gelu,
                                             bias=c["b1"][:, l, half:half + 1])
                        f1[half] = fh
                    ps_f2 = pp.tile([64, M], F32, tag="big", name=_nm("psf2"), bufs=2)
                    nc.tensor.matmul(ps_f2, c["w2"][:, l, 0, :], f1[0], start=True, stop=False)
                    nc.tensor.matmul(ps_f2, c["w2"][:, l, 1, :], f1[1], start=False, stop=True)
                    f2 = wk.tile([64, M], F32, tag="f2", name=_nm("f2"))
                    nc.vector.tensor_scalar(out=f2, in0=ps_f2,
                                            scalar1=sh64[:, base + 3:base + 4],
                                            scalar2=None, op0=OP.add)
                    e2 = wk.tile([64, M], F32, tag="e2", name=_nm("e2"))
                    nc.vector.tensor_add(out=e2, in0=eL, in1=f2)
                    eT = _emit_ln(nc, wk, pp, e2, M, sh64[:, base + 4:base + 5],
                                  sh64[:, base + 5:base + 6], False,
                                  c["ones64"], c["ones164"], eps1)
                    dbg(f"eT_l{l}", eT[:], [64, M])

                # ---- heads ----
                ps_eh = pp.tile([2, M], F32, tag="big", name=_nm("pseh"), bufs=2)
                nc.tensor.matmul(ps_eh, c["whe"], eT, start=True, stop=True)
                eh = wk.tile([2, M], F32, tag="eh", name="eh")
                nc.vector.tensor_scalar(out=eh, in0=ps_eh, scalar1=sh64[0:2, 21:22],
                                        scalar2=None, op0=OP.add)
                nc.sync.dma_start(out=edge_out[:], in_=eh)

                NV = P_COLS - 2
                ps_n = pp.tile([64, 2 * NV], F32, tag="big", name=_nm("psn"), bufs=2)
                eTr = eT[:].rearrange("c (q r) -> c q r", r=11)
                for b in range(BPC):
                    slices = [
                        eTr[:, 12 * b + 2:12 * b + 12, 0],      # VX: cols 11v
                        eTr[:, 12 * b + 2:12 * b + 12, 1],      # VY: cols 11v+1
                        eT[:, E * b + 1:E * b + 11],            # XV: cols v-1
                        eT[:, E * b + 12:E * b + 22],           # YV: cols v+10
                    ]
                    for gdx, sl in enumerate(slices):
                        nc.tensor.matmul(ps_n[:, b * NV:(b + 1) * NV], c["wn"][:, gdx, :],
                                         sl, start=(gdx == 0), stop=(gdx == 3))
                n_sb = wk.tile([64, 2 * NV], F32, tag="nsb", name="nsb")
                nc.vector.tensor_scalar(out=n_sb, in0=ps_n, scalar1=sh64[:, 22:23],
                                        scalar2=None, op0=OP.add)
                ne = _emit_ln(nc, wk, pp, n_sb, 2 * NV, sh64[:, 23:24], sh64[:, 24:25],
                              True, c["ones64"], c["ones164"], eps1)
                ps_nh = pp.tile([8, 2 * NV], F32, tag="big", name=_nm("psnh"), bufs=2)
                nc.tensor.matmul(ps_nh, c["wnh"], ne, start=True, stop=True)
                nh = wk.tile([8, 2 * NV], F32, tag="nh", name="nh")
                nc.vector.tensor_scalar(out=nh, in0=ps_nh, scalar1=sh64[0:8, 25:26],
                                        scalar2=None, op0=OP.add)
                nc.sync.dma_start(out=node_out[:], in_=nh)

    nc.compile()
    return nc


def prep_params(params):
    """Host-side packing of all replicated weights/constants."""
    g = lambda *ks: np.asarray(_dig(params, ks), np.float32)
    pp = {}
    # stem: w [8,64], b [64]
    ws = g("stem", "w")
    wstem = np.zeros((128, 64), np.float32)
    wstem[0:8] = ws
    wstem[64:72] = ws
    pp["wstem"] = wstem.astype(BF)
    bs = g("stem", "b")
    pp["bstem"] = np.tile(bs, 2).reshape(128, 1).astype(np.float32)
    # conv: w [64,64,3] per layer -> [128(i dup), 5, 3, 64(o)]
    wc = np.zeros((128, NL, 3, 64), np.float32)
    gn = np.zeros((128, NL, 2), np.float32)
    for l, cb in enumerate(params["conv"]):
        w = np.asarray(cb["w"], np.float32)          # [o, i, t]
        wt = np.transpose(w, (1, 2, 0))              # [i, t, o]
        wc[0:64, l] = wt
        wc[64:128, l] = wt
        gn[0:64, l, 0] = np.asarray(cb["gn_g"], np.float32)
        gn[64:128, l, 0] = gn[0:64, l, 0]
        gn[0:64, l, 1] = np.asarray(cb["gn_b"], np.float32)
        gn[64:128, l, 1] = gn[0:64, l, 1]
    pp["wconv"] = wc.astype(BF)
    pp["gnw"] = gn
    # group-stat matmul helpers
    g8 = np.zeros((128, 16), np.float32)
    for p in range(128):
        g8[p, p // 8] = 1.0 / 8.0
    pp["g8"] = g8
    g8t = np.zeros((16, 128), np.float32)
    for p in range(128):
        g8t[p // 8, p] = 1.0
    pp["g8t"] = g8t
    pp["iota7"] = np.arange(7, dtype=np.float32).reshape(7, 1)
    pp["te"] = g("type_emb")
    # edge merge: lin.w [128, 64] -> [64, 2, 64] with conv part scaled by 1/512
    wm = g("edge_merge", "lin", "w")
    wmp = np.zeros((64, 2, 64), np.float32)
    wmp[:, 0, :] = wm[0:64] / float(N)
    wmp[:, 1, :] = wm[64:128]
    pp["wm"] = wmp
    pp["ones64"] = np.full((64, 1), 1.0 / 64.0, np.float32)
    pp["ones164"] = np.ones((1, 64), np.float32)
    pp["id128"] = np.eye(128, dtype=np.float32)
    sel4 = np.zeros((128, 16), np.float32)
    sel4[:, [4 * h + h for h in range(4)]] = 1.0
    pp["sel4"] = sel4
    ind4 = np.zeros((4, 128), np.float32)
    for h in range(4):
        ind4[h, 32 * h:32 * h + 16] = 1.0
    pp["ind4"] = ind4
    # attention weights (padded head-strided layout)
    wqkv = np.zeros((64, 6, 128), np.float32)
    bqkv = np.zeros((128, 6, 1), np.float32)
    wo = np.zeros((128, 2, 64), np.float32)
    w1 = np.zeros((64, 2, 256), np.float32)
    b1 = np.zeros((128, 2, 2), np.float32)
    w2 = np.zeros((128, 2, 2, 64), np.float32)
    polyco = np.zeros((2, 4, 6), np.float64)
    for l, ap in enumerate(params["attn"]):
        for j, nm_ in enumerate(["q", "k", "v"]):
            w = np.asarray(ap[nm_]["w"], np.float32)
            bb = np.asarray(ap[nm_]["b"], np.float32)
            if nm_ == "q":
                w = w * SCALE
                bb = bb * SCALE
            for h in range(4):
                wqkv[:, l * 3 + j, 32 * h:32 * h + 16] = w[:, 16 * h:16 * h + 16]
                bqkv[32 * h:32 * h + 16, l * 3 + j, 0] = bb[16 * h:16 * h + 16]
        won = np.asarray(ap["o"]["w"], np.float32)
        for h in range(4):
            wo[32 * h:32 * h + 16, l, :] = won[16 * h:16 * h + 16, :]
        w1[:, l, :] = np.asarray(ap["ff1"]["w"], np.float32)
        b1f = np.asarray(ap["ff1"]["b"], np.float32)
        b1[:, l, 0] = b1f[0:128]
        b1[:, l, 1] = b1f[128:256]
        w2f = np.asarray(ap["ff2"]["w"], np.float32)
        w2[:, l, 0, :] = w2f[0:128]
        w2[:, l, 1, :] = w2f[128:256]
        sbv = np.asarray(ap["struct_bias"], np.float64)   # [6, 4]
        for h in range(4):
            polyco[l, h] = _lagrange6(sbv[:, h])
    pp["wqkv"] = wqkv
    pp["bqkv"] = bqkv
    pp["wo"] = wo
    pp["w1"] = w1
    pp["b1"] = b1
    pp["w2"] = w2
    pp["polyco"] = polyco
    pp["whe"] = g("edge_head", "w")
    pp["wn"] = np.transpose(g("node_merge", "lin", "w").reshape(4, 64, 64), (1, 0, 2)).copy()
    pp["wnh"] = g("node_head", "w")
    # sheet64: misc per-partition scalars
    sh = np.zeros((64, 26), np.float32)
    sh[:, 0] = g("edge_merge", "lin", "b")
    sh[:, 1] = g("edge_merge", "ln_g")
    sh[:, 2] = g("edge_merge", "ln_b")
    for l, ap in enumerate(params["attn"]):
        base = 3 + l * 9
        sh[:, base + 0] = np.asarray(ap["o"]["b"], np.float32)
        sh[:, base + 1] = np.asarray(ap["ln1_g"], np.float32)
        sh[:, base + 2] = np.asarray(ap["ln1_b"], np.float32)
        sh[:, base + 3] = np.asarray(ap["ff2"]["b"], np.float32)
        sh[:, base + 4] = np.asarray(ap["ln2_g"], np.float32)
        sh[:, base + 5] = np.asarray(ap["ln2_b"], np.float32)
    sh[0:2, 21] = g("edge_head", "b")
    sh[:, 22] = g("node_merge", "lin", "b")
    sh[:, 23] = g("node_merge", "ln_g")
    sh[:, 24] = g("node_merge", "ln_b")
    sh[0:8, 25] = g("node_head", "b")
    pp["sheet64"] = sh
    return pp


def _dig(d, ks):
    for k in ks:
        d = d[k]
    return d


_CACHE = {}


def kernel(edge_data, edge_types, edge_mask, struct_rel, params):
    edge_data = np.asarray(edge_data, np.float32)
    edge_types = np.asarray(edge_types)
    edge_mask = np.asarray(edge_mask)
    struct_rel = np.asarray(struct_rel)

    pp = prep_params(params)
    key = "nc"
    if key not in _CACHE:
        _CACHE[key] = build_nc(pp)
    nc = _CACHE[key]

    in_maps = []
    for cidx in range(NCORES):
        b0 = cidx * BPC
        sl = slice(b0, b0 + BPC)
        m = {k: v for k, v in pp.items() if k != "polyco"}
        m["edb"] = edge_data[sl].reshape(M, C_IN, N).astype(BF)
        m["typesf"] = edge_types[sl].reshape(1, M).astype(np.float32)
        m["maskk"] = edge_mask[sl].reshape(BPC, E, 1).astype(np.float32)
        m["relt"] = np.ascontiguousarray(
            np.transpose(struct_rel[sl], (0, 2, 1))).astype(np.float32)
        in_maps.append(m)

    res = run_bass_kernel_spmd(nc, in_maps, list(range(NCORES)))

    el = np.zeros((B, E, 2), np.float32)
    nl = np.zeros((B, P_COLS - 2, 8), np.float32)
    for cidx in range(NCORES):
        r = res.results[cidx]
        eo = r["edge_out"].reshape(2, BPC, E).transpose(1, 2, 0)
        no = r["node_out"].reshape(8, BPC, P_COLS - 2).transpose(1, 2, 0)
        el[cidx * BPC:(cidx + 1) * BPC] = eo
        nl[cidx * BPC:(cidx + 1) * BPC] = no
    return el, nl
